# revision 4
# baseline (speedup 1.0000x reference)
"""Trainium2 Bass kernel for 2-layer GraphSAGE (mean aggregation).

Strategy (8-core SPMD, nodes sharded 12500/core):
- Host: sorts/pads each core's in-edges into fixed 128-edge tiles aligned to
  128-dst-node windows (uniform tile count across cores so one SPMD program
  works), pre-gathers layer-1 messages x[src] (input reindexing) and bakes
  1/deg into per-edge weights.
- Device layer 1: stream pre-gathered messages, segment-sum via one-hot
  indicator matmuls (M[e,r] = (dst_e==r)*w_e built on DVE from an iota tile),
  PSUM-accumulated per 512-node window, then W1l/W1r matmuls + bias + ReLU in
  [feat, node] orientation.
- h1 transposed to row layout via PE-identity matmuls, AllGather -> full
  [100352, 64] bf16 table per core.
- Device layer 2: per-tile indirect-DMA gather of h1 rows, same one-hot
  aggregation, W2l/W2r matmuls + bias, transpose back, DMA out fp32.
"""
import sys

sys.path.insert(0, '/opt/trn_rl_repo')
import numpy as np
import ml_dtypes

BF16 = ml_dtypes.bfloat16
N = 100000
D = 64
NCORES = 8
NLOC = N // NCORES          # 12500
P = 128
NW = (NLOC + P - 1) // P    # 98 dst windows per core
WROWS = NW * P              # 12544 padded local rows
TBL_ROWS = NCORES * WROWS   # 100352 rows in the gathered h1 table


def _layout_row(n):
    """Row index of global node n inside the AllGather'd h1 table."""
    c = n // NLOC
    r = n % NLOC
    t = r // P
    p = r % P
    return c * WROWS + p * NW + t


def _prep_core(c, src, dst, inv, x, K):
    """Slot edges of core c into NW*K tiles of 128, window-aligned."""
    m = (dst >= c * NLOC) & (dst < (c + 1) * NLOC)
    es, ed = src[m], dst[m] - c * NLOC
    w = inv[dst[m]]
    win = ed // P
    order = np.argsort(win, kind='stable')
    es, ed, w, win = es[order], ed[order], w[order], win[order]

    T = NW * K
    slots_src = np.zeros(T * P, dtype=np.int64)
    slots_dstloc = np.full(T * P, -1.0, dtype=np.float32)
    slots_w = np.zeros(T * P, dtype=np.float32)
    # fill window-by-window
    counts = np.bincount(win, minlength=NW)
    starts = np.concatenate([[0], np.cumsum(counts)[:-1]])
    for wi in range(NW):
        cnt = counts[wi]
        base = wi * K * P
        sl = slice(starts[wi], starts[wi] + cnt)
        slots_src[base:base + cnt] = es[sl]
        slots_dstloc[base:base + cnt] = (ed[sl] % P).astype(np.float32)
        slots_w[base:base + cnt] = w[sl]

    # [T*P] slot-major (tile t, partition p = slot t*P+p) -> [P, T] arrays
    def to_pt(a, dt):
        return np.ascontiguousarray(a.reshape(T, P).T.astype(dt))

    dstloc_pt = to_pt(slots_dstloc, np.float32)
    w_pt = to_pt(slots_w, np.float32)
    src2_pt = to_pt(_layout_row(slots_src), np.int32)
    # pre-gathered layer-1 messages, bf16, [P, T*64] partition-major
    msgs = x[slots_src].astype(BF16)           # [T*P, 64]
    msgs_pt = np.ascontiguousarray(
        msgs.reshape(T, P, D).transpose(1, 0, 2).reshape(P, T * D))
    # local x^T padded to WROWS cols
    xT = np.zeros((D, WROWS), dtype=BF16)
    xT[:, :NLOC] = x[c * NLOC:(c + 1) * NLOC].T.astype(BF16)
    return msgs_pt, dstloc_pt, w_pt, src2_pt, xT


def _build_program(K, reps=1):
    import concourse.bass as bass
    import concourse.tile as tile
    from concourse import bacc, mybir

    T = NW * K
    nc = bacc.Bacc("TRN2", target_bir_lowering=False, debug=False,
                   num_devices=NCORES)
    dt = mybir.dt

    msgs_d = nc.dram_tensor("msgs", [P, T * D], dt.bfloat16, kind="ExternalInput")
    dstloc_d = nc.dram_tensor("dstloc", [P, T], dt.float32, kind="ExternalInput")
    wts_d = nc.dram_tensor("wts", [P, T], dt.float32, kind="ExternalInput")
    src2_d = nc.dram_tensor("src2", [P, T], dt.int32, kind="ExternalInput")
    xT_d = nc.dram_tensor("xT", [D, WROWS], dt.bfloat16, kind="ExternalInput")
    iota_d = nc.dram_tensor("iota", [P, P], dt.bfloat16, kind="ExternalInput")
    id64_d = nc.dram_tensor("id64", [D, D], dt.bfloat16, kind="ExternalInput")
    id64f_d = nc.dram_tensor("id64f", [D, D], dt.float32, kind="ExternalInput")
    w1l_d = nc.dram_tensor("w1lT", [D, D], dt.bfloat16, kind="ExternalInput")
    w1r_d = nc.dram_tensor("w1rT", [D, D], dt.bfloat16, kind="ExternalInput")
    w2l_d = nc.dram_tensor("w2lT", [D, D], dt.bfloat16, kind="ExternalInput")
    w2r_d = nc.dram_tensor("w2rT", [D, D], dt.bfloat16, kind="ExternalInput")
    b1_d = nc.dram_tensor("b1c", [D, 1], dt.float32, kind="ExternalInput")
    b2_d = nc.dram_tensor("b2c", [D, 1], dt.float32, kind="ExternalInput")
    out_d = nc.dram_tensor("out", [WROWS, D], dt.float32, kind="ExternalOutput")

    # supers: groups of up to 4 windows sharing one [64,512] psum bank
    supers = []
    wi = 0
    while wi < NW:
        sw = min(4, NW - wi)
        supers.append((wi, sw))
        wi += sw

    CHUNK_W = 14  # windows of msgs per streamed chunk
    with tile.TileContext(nc) as tc:
        with (
            tc.tile_pool(name="const", bufs=1) as cpool,
            tc.tile_pool(name="chunks", bufs=2) as chpool,
            tc.tile_pool(name="mtiles", bufs=8) as mpool,
            tc.tile_pool(name="gtiles", bufs=12) as gpool,
            tc.tile_pool(name="small", bufs=3) as spool,
            tc.tile_pool(name="psA", bufs=2, space="PSUM") as psA,
            tc.tile_pool(name="psB", bufs=2, space="PSUM") as psB,
            tc.tile_pool(name="psT", bufs=2, space="PSUM") as psT,
            tc.tile_pool(name="dram", bufs=1, space="DRAM") as dpool,
        ):
            # resident SBUF state
            dstloc_sb = cpool.tile([P, T], dt.float32, tag="dstloc")
            wts_sb = cpool.tile([P, T], dt.float32, tag="wts")
            src2_sb = cpool.tile([P, T], dt.int32, tag="src2")
            xT_sb = cpool.tile([D, WROWS], dt.bfloat16, tag="xT")
            iota_sb = cpool.tile([P, P], dt.bfloat16, tag="iota")
            id64_sb = cpool.tile([D, D], dt.bfloat16, tag="id64")
            id64f_sb = cpool.tile([D, D], dt.float32, tag="id64f")
            w1l_sb = cpool.tile([D, D], dt.bfloat16, tag="w1l")
            w1r_sb = cpool.tile([D, D], dt.bfloat16, tag="w1r")
            w2l_sb = cpool.tile([D, D], dt.bfloat16, tag="w2l")
            w2r_sb = cpool.tile([D, D], dt.bfloat16, tag="w2r")
            b1_sb = cpool.tile([D, 1], dt.float32, tag="b1")
            b2_sb = cpool.tile([D, 1], dt.float32, tag="b2")
            h1T_sb = cpool.tile([D, WROWS], dt.bfloat16, tag="h1T")
            h1rows_sb = cpool.tile([P, NW * D], dt.bfloat16, tag="h1rows")

            for t_sb, t_d in [(dstloc_sb, dstloc_d), (wts_sb, wts_d),
                              (src2_sb, src2_d), (xT_sb, xT_d),
                              (iota_sb, iota_d), (id64_sb, id64_d),
                              (id64f_sb, id64f_d),
                              (w1l_sb, w1l_d), (w1r_sb, w1r_d),
                              (w2l_sb, w2l_d), (w2r_sb, w2r_d),
                              (b1_sb, b1_d), (b2_sb, b2_d)]:
                nc.sync.dma_start(out=t_sb[:], in_=t_d.ap())

            h1loc_dram = dpool.tile([WROWS, D], dt.bfloat16, tag="h1loc")
            h1full_dram = dpool.tile([TBL_ROWS, D], dt.bfloat16, tag="h1full")

            for _rep in range(reps):
                # ---------------- layer 1 ----------------
                nchunks = (NW + CHUNK_W - 1) // CHUNK_W
                chunk_tiles = {}
                for ci in range(nchunks):
                    w0 = ci * CHUNK_W
                    nw = min(CHUNK_W, NW - w0)
                    ch = chpool.tile([P, CHUNK_W * K * D], dt.bfloat16, tag="msgs")
                    nc.sync.dma_start(
                        out=ch[:, :nw * K * D],
                        in_=msgs_d.ap()[:, w0 * K * D:(w0 + nw) * K * D])
                    chunk_tiles[ci] = ch

                for w0, sw in supers:
                    agg_ps = psA.tile([D, 4 * P], dt.float32, tag="agg")
                    for s in range(sw):
                        wi = w0 + s
                        ci, woff = wi // CHUNK_W, wi % CHUNK_W
                        ch = chunk_tiles[ci]
                        for k in range(K):
                            t = wi * K + k
                            mt = mpool.tile([P, P], dt.bfloat16, tag="M")
                            nc.vector.tensor_scalar(
                                out=mt[:], in0=iota_sb[:],
                                scalar1=dstloc_sb[:, t:t + 1],
                                scalar2=wts_sb[:, t:t + 1],
                                op0=mybir.AluOpType.is_equal,
                                op1=mybir.AluOpType.mult)
                            nc.tensor.matmul(
                                out=agg_ps[:, s * P:(s + 1) * P],
                                lhsT=ch[:, (woff * K + k) * D:(woff * K + k + 1) * D],
                                rhs=mt[:], start=(k == 0), stop=(k == K - 1))
                    agg_sb = spool.tile([D, 4 * P], dt.bfloat16, tag="aggsb")
                    nc.vector.tensor_copy(out=agg_sb[:, :sw * P],
                                          in_=agg_ps[:, :sw * P])
                    h_ps = psB.tile([D, 4 * P], dt.float32, tag="hps")
                    nc.tensor.matmul(out=h_ps[:, :sw * P], lhsT=w1l_sb[:],
                                     rhs=agg_sb[:, :sw * P], start=True, stop=False)
                    nc.tensor.matmul(out=h_ps[:, :sw * P], lhsT=w1r_sb[:],
                                     rhs=xT_sb[:, w0 * P:(w0 + sw) * P],
                                     start=False, stop=True)
                    nc.scalar.activation(
                        out=h1T_sb[:, w0 * P:(w0 + sw) * P], in_=h_ps[:, :sw * P],
                        func=mybir.ActivationFunctionType.Relu, bias=b1_sb[:])

                # transpose h1T -> rows, stage, one DMA to DRAM, AllGather
                for wi in range(NW):
                    tp = psT.tile([P, D], dt.float32, tag="tp")
                    nc.tensor.matmul(out=tp[:], lhsT=h1T_sb[:, wi * P:(wi + 1) * P],
                                     rhs=id64_sb[:], start=True, stop=True)
                    nc.vector.tensor_copy(out=h1rows_sb[:, wi * D:(wi + 1) * D],
                                          in_=tp[:])
                nc.sync.dma_start(
                    out=h1loc_dram[:].rearrange("(p t) f -> p (t f)", p=P),
                    in_=h1rows_sb[:])
                nc.gpsimd.collective_compute(
                    "AllGather", mybir.AluOpType.bypass,
                    replica_groups=[list(range(NCORES))],
                    ins=[h1loc_dram[:]], outs=[h1full_dram[:]])

                # ---------------- layer 2 ----------------
                out2T_sb = cpool.tile([D, WROWS], dt.float32, tag="out2T")
                for w0, sw in supers:
                    agg_ps = psA.tile([D, 4 * P], dt.float32, tag="agg")
                    for s in range(sw):
                        wi = w0 + s
                        for k in range(K):
                            t = wi * K + k
                            gt = gpool.tile([P, D], dt.bfloat16, tag="g")
                            nc.gpsimd.indirect_dma_start(
                                out=gt[:], out_offset=None, in_=h1full_dram[:],
                                in_offset=bass.IndirectOffsetOnAxis(
                                    ap=src2_sb[:, t:t + 1], axis=0))
                            mt = mpool.tile([P, P], dt.bfloat16, tag="M")
                            nc.vector.tensor_scalar(
                                out=mt[:], in0=iota_sb[:],
                                scalar1=dstloc_sb[:, t:t + 1],
                                scalar2=wts_sb[:, t:t + 1],
                                op0=mybir.AluOpType.is_equal,
                                op1=mybir.AluOpType.mult)
                            nc.tensor.matmul(
                                out=agg_ps[:, s * P:(s + 1) * P], lhsT=gt[:],
                                rhs=mt[:], start=(k == 0), stop=(k == K - 1))
                    agg_sb = spool.tile([D, 4 * P], dt.bfloat16, tag="aggsb")
                    nc.vector.tensor_copy(out=agg_sb[:, :sw * P],
                                          in_=agg_ps[:, :sw * P])
                    h_ps = psB.tile([D, 4 * P], dt.float32, tag="hps")
                    nc.tensor.matmul(out=h_ps[:, :sw * P], lhsT=w2l_sb[:],
                                     rhs=agg_sb[:, :sw * P], start=True, stop=False)
                    nc.tensor.matmul(out=h_ps[:, :sw * P], lhsT=w2r_sb[:],
                                     rhs=h1T_sb[:, w0 * P:(w0 + sw) * P],
                                     start=False, stop=True)
                    nc.vector.tensor_scalar_add(
                        out=out2T_sb[:, w0 * P:(w0 + sw) * P],
                        in0=h_ps[:, :sw * P], scalar1=b2_sb[:])

                # transpose out2T -> rows -> DRAM
                outv = out_d.ap().rearrange("(p t) f -> p t f", p=P)
                for wi in range(NW):
                    tp = psT.tile([P, D], dt.float32, tag="tp")
                    nc.tensor.matmul(out=tp[:], lhsT=out2T_sb[:, wi * P:(wi + 1) * P],
                                     rhs=id64f_sb[:], start=True, stop=True)
                    ot = spool.tile([P, D], dt.float32, tag="orow")
                    nc.vector.tensor_copy(out=ot[:], in_=tp[:])
                    nc.sync.dma_start(out=outv[:, wi, :], in_=ot[:])

    nc.compile()
    return nc


PREP_VERSION = 2


def prep_all(inputs):
    """Host-side prep: returns (K, in_maps) for the 8 cores."""
    x = np.asarray(inputs["x"], dtype=np.float32)
    edge_index = np.asarray(inputs["edge_index"])
    src = edge_index[0].astype(np.int64)
    dst = edge_index[1].astype(np.int64)
    cnt = np.bincount(dst, minlength=N).astype(np.float32)
    inv = (1.0 / np.maximum(cnt, 1.0)).astype(np.float32)

    # uniform tile count across cores/windows
    wid = (dst % NLOC) // P + (dst // NLOC) * NW
    wc = np.bincount(wid, minlength=NCORES * NW)
    K = int(np.max((wc + P - 1) // P))
    K = max(K, 1)

    iota = np.tile(np.arange(P, dtype=np.float32), (P, 1)).astype(BF16)
    id64 = np.eye(D, dtype=np.float32)
    common = {
        "iota": iota, "id64": id64.astype(BF16), "id64f": id64,
        "w1lT": np.asarray(inputs["W1l"], np.float32).T.astype(BF16).copy(),
        "w1rT": np.asarray(inputs["W1r"], np.float32).T.astype(BF16).copy(),
        "w2lT": np.asarray(inputs["W2l"], np.float32).T.astype(BF16).copy(),
        "w2rT": np.asarray(inputs["W2r"], np.float32).T.astype(BF16).copy(),
        "b1c": np.asarray(inputs["b1"], np.float32).reshape(D, 1).copy(),
        "b2c": np.asarray(inputs["b2"], np.float32).reshape(D, 1).copy(),
    }
    in_maps = []
    for c in range(NCORES):
        msgs_pt, dstloc_pt, w_pt, src2_pt, xT = _prep_core(
            c, src, dst, inv, x, K)
        in_maps.append({**common, "msgs": msgs_pt, "dstloc": dstloc_pt,
                        "wts": w_pt, "src2": src2_pt, "xT": xT})
    return K, in_maps


def kernel(x, edge_index, W1l, W1r, b1, W2l, W2r, b2):
    from concourse import bass_utils

    K, in_maps = prep_all(dict(x=x, edge_index=edge_index, W1l=W1l, W1r=W1r,
                               b1=b1, W2l=W2l, W2r=W2r, b2=b2))
    nc = _build_program(K)
    res = bass_utils.run_bass_kernel_spmd(nc, in_maps, list(range(NCORES)))

    outs = []
    for c in range(NCORES):
        o = res.results[c]["out"]  # [WROWS, 64], row = p*NW + t
        o = o.reshape(P, NW, D).transpose(1, 0, 2).reshape(WROWS, D)[:NLOC]
        outs.append(o)
    return np.concatenate(outs, axis=0).astype(np.float32)



# revision 16
# speedup vs baseline: 2.2575x; 2.2575x over previous
"""Trainium2 Bass kernel for 2-layer GraphSAGE (mean aggregation).

Strategy (8-core SPMD, nodes sharded 12500/core):
- Host: slots each core's in-edges into fixed 128-edge tiles aligned to
  128-dst-node windows, pre-gathers layer-1 messages x[src] and bakes 1/deg
  into per-edge weights.
- Device layer 1: stream pre-gathered messages, segment-sum via one-hot
  indicator matmuls (M[e,r] = (dst_e==r)*w_e built on DVE from an iota tile),
  PSUM-accumulated per 512-node window, then W1l/W1r matmuls + bias + ReLU in
  [feat, node] orientation.
- h1 transposed to row layout (rows padded to 128 feats so each row is 256B),
  AllGather -> full [100352, 128] bf16 table per core.
- Device layer 2: tiles additionally grouped by table QUARTER (25088 rows =
  2 src cores) so row indices fit int16; h1 rows fetched with large batched
  dma_gather ops (CounterMachine path, ~1us + 0.34ns/desc instead of ~1.4us
  per 128-row indirect DMA), same one-hot aggregation, W2l/W2r matmuls +
  bias, transpose back, DMA out fp32.
"""
import sys

sys.path.insert(0, '/opt/trn_rl_repo')
import numpy as np
import ml_dtypes

BF16 = ml_dtypes.bfloat16
N = 100000
D = 64
E2 = 128                     # padded table row width (256B rows)
NCORES = 8
NLOC = N // NCORES           # 12500
P = 128
NW = (NLOC + P - 1) // P     # 98 dst windows per core
WROWS = NW * P               # 12544 padded local rows
TBL_ROWS = NCORES * WROWS    # 100352 rows in the gathered h1 table
NQ = 4                       # table quarters (int16 index ranges)
QROWS = TBL_ROWS // NQ       # 25088 rows per quarter
CW1 = 14                     # layer-1 msg chunk width (windows)
CW2 = 7                      # layer-2 gather chunk width (windows)


def _layout_row(n):
    """Row index of global node n inside the AllGather'd h1 table."""
    c = n // NLOC
    r = n % NLOC
    t = r // P
    p = r % P
    return c * WROWS + p * NW + t


def _wrap16(flat_idx):
    """int16 index stream -> [16, n/16] wrapped, replicated to 128 parts."""
    ni = flat_idx.shape[0]
    assert ni % 16 == 0
    w = np.zeros((16, ni // 16), np.int16)
    w[np.arange(ni) % 16, np.arange(ni) // 16] = flat_idx
    return np.tile(w, (8, 1))


def _prep_core(c, src, dst, inv, x, K, K2):
    """Slot edges of core c.

    Layer 1: (w, k, p) layout, K tiles/window; pre-gathered msgs.
    Layer 2: (chunk, q, w, k, p) layout, K2[q] tiles per (window, quarter);
    int16 gather index streams per (chunk, q).
    """
    m = (dst >= c * NLOC) & (dst < (c + 1) * NLOC)
    es, ed = src[m], dst[m] - c * NLOC
    w = inv[dst[m]]
    win = ed // P
    row = _layout_row(es)

    # ---------- layer 1 ----------
    order = np.argsort(win, kind='stable')
    es1, ed1, w1 = es[order], ed[order], w[order]
    win1 = win[order]
    T = NW * K
    slots_src = np.zeros(T * P, dtype=np.int64)
    slots_dstloc = np.full(T * P, -1.0, dtype=np.float32)
    slots_w = np.zeros(T * P, dtype=np.float32)
    counts = np.bincount(win1, minlength=NW)
    starts = np.concatenate([[0], np.cumsum(counts)[:-1]])
    for wi in range(NW):
        cnt = counts[wi]
        base = wi * K * P
        sl = slice(starts[wi], starts[wi] + cnt)
        slots_src[base:base + cnt] = es1[sl]
        slots_dstloc[base:base + cnt] = (ed1[sl] % P).astype(np.float32)
        slots_w[base:base + cnt] = w1[sl]

    def to_pt(a, dt):
        T_ = a.shape[0] // P
        return np.ascontiguousarray(a.reshape(T_, P).T.astype(dt))

    dstloc1 = to_pt(slots_dstloc, np.float32)
    wts1 = to_pt(slots_w, np.float32)
    msgs = x[slots_src].astype(BF16)           # [T*P, 64]
    msgs_pt = np.ascontiguousarray(
        msgs.reshape(T, P, D).transpose(1, 0, 2).reshape(P, T * D))

    # ---------- layer 2 ----------
    q = row // QROWS
    order2 = np.argsort(win * NQ + q, kind='stable')
    ed2, w2 = ed[order2], w[order2]
    row2, q2 = row[order2], q[order2]
    win2 = win[order2]
    sumK2 = int(sum(K2))
    pref = np.concatenate([[0], np.cumsum(K2)]).astype(int)
    T2 = NW * sumK2
    sl_dstloc = np.full(T2 * P, -1.0, dtype=np.float32)
    sl_w = np.zeros(T2 * P, dtype=np.float32)
    sl_row = np.zeros(T2 * P, dtype=np.int64)   # quarter-local table row
    gcounts = np.bincount(win2 * NQ + q2, minlength=NW * NQ).reshape(NW, NQ)
    gstarts = np.concatenate([[0], np.cumsum(gcounts.flatten())[:-1]]
                             ).reshape(NW, NQ)
    for wi in range(NW):
        ci, wl = wi // CW2, wi % CW2
        for qi in range(NQ):
            cnt = gcounts[wi, qi]
            # tile index in (chunk, q, w_local, k) order
            t0 = (ci * sumK2 * CW2 + pref[qi] * CW2 + wl * K2[qi])
            base = t0 * P
            sl = slice(gstarts[wi, qi], gstarts[wi, qi] + cnt)
            assert cnt <= K2[qi] * P
            sl_dstloc[base:base + cnt] = (ed2[sl] % P).astype(np.float32)
            sl_w[base:base + cnt] = w2[sl]
            sl_row[base:base + cnt] = row2[sl] - qi * QROWS
            # pad slots keep row 0 (valid), weight 0
    dstloc2 = to_pt(sl_dstloc, np.float32)
    wts2 = to_pt(sl_w, np.float32)
    # gather index streams: per (chunk, q) contiguous, slot order (w, k, p)
    idx2 = _wrap16(sl_row.astype(np.int16))     # [128, T2*8]

    # local x^T padded to WROWS cols
    xT = np.zeros((D, WROWS), dtype=BF16)
    xT[:, :NLOC] = x[c * NLOC:(c + 1) * NLOC].T.astype(BF16)
    return dict(msgs=msgs_pt, dstloc1=dstloc1, wts1=wts1,
                dstloc2=dstloc2, wts2=wts2, idx2=idx2, xT=xT)


def _gather_direct(nc, mybir, out_ap, in_ap, idxs_ap, num_idxs, elem_size,
                   elem_step, queue_num=0):
    """InstDMAGatherAnt with elem_size_bytes=128 (stride stays 256B-aligned);
    bass.dma_gather asserts elem%256B although transpose=False handles 128B."""
    eng = nc.gpsimd
    dtype_size = mybir.dt.size(in_ap.dtype)
    stride_bytes_256 = (elem_step * dtype_size) // 256
    _in_ap = eng.lower_ap_dma(in_ap, for_custom_bir_dma=True)
    _idxs_ap = eng.lower_ap(idxs_ap)
    _out_ap = eng.lower_ap(out_ap)
    return eng.add_instruction(
        mybir.InstDMAGatherAnt(
            name=nc.get_next_instruction_name(),
            ins=[*_in_ap, _idxs_ap, eng.lower_val_access(eng.to_reg(num_idxs))],
            outs=[_out_ap],
            transpose=False, num_idxs=num_idxs, elem_size=elem_size,
            stride_bytes_256=stride_bytes_256, gen_mode=0, single_packet=True,
            queue_num=queue_num, sbuf_tokens_per_rank=0,
            sbuf_free_dim_per_rank=0, sbuf_free_dim_pad_per_rank=0,
            sbuf_byte_offset=0))


def _build_program(K, K2, reps=1, probe=()):
    import concourse.bass as bass
    import concourse.tile as tile
    from concourse import bacc, mybir

    probe = set(probe)
    GELEM = E2 if "g128" in probe else 64
    T = NW * K
    sumK2 = int(sum(K2))
    pref = [0]
    for k2 in K2:
        pref.append(pref[-1] + k2)
    T2 = NW * sumK2
    nchunks2 = NW // CW2
    CH_COLS1 = CW1 * K * D            # layer-1 msg cols per chunk
    CH_COLS2 = CW2 * sumK2 * GELEM    # layer-2 gathered cols per chunk
    CH_COLS = max(CH_COLS1, CH_COLS2)

    nc = bacc.Bacc("TRN2", target_bir_lowering=False, debug=False,
                   num_devices=NCORES)
    dt = mybir.dt

    msgs_d = nc.dram_tensor("msgs", [P, T * D], dt.bfloat16, kind="ExternalInput")
    dstloc1_d = nc.dram_tensor("dstloc1", [P, T], dt.float32, kind="ExternalInput")
    wts1_d = nc.dram_tensor("wts1", [P, T], dt.float32, kind="ExternalInput")
    dstloc2_d = nc.dram_tensor("dstloc2", [P, T2], dt.float32, kind="ExternalInput")
    wts2_d = nc.dram_tensor("wts2", [P, T2], dt.float32, kind="ExternalInput")
    idx2_d = nc.dram_tensor("idx2", [P, T2 * 8], dt.int16, kind="ExternalInput")
    xT_d = nc.dram_tensor("xT", [D, WROWS], dt.bfloat16, kind="ExternalInput")
    iota_d = nc.dram_tensor("iota", [P, P], dt.bfloat16, kind="ExternalInput")
    id64_d = nc.dram_tensor("id64", [D, D], dt.bfloat16, kind="ExternalInput")
    id64f_d = nc.dram_tensor("id64f", [D, D], dt.float32, kind="ExternalInput")
    w1l_d = nc.dram_tensor("w1lT", [D, D], dt.bfloat16, kind="ExternalInput")
    w1r_d = nc.dram_tensor("w1rT", [D, D], dt.bfloat16, kind="ExternalInput")
    w2l_d = nc.dram_tensor("w2lT", [D, D], dt.bfloat16, kind="ExternalInput")
    w2r_d = nc.dram_tensor("w2rT", [D, D], dt.bfloat16, kind="ExternalInput")
    b1_d = nc.dram_tensor("b1c", [D, 1], dt.float32, kind="ExternalInput")
    b2_d = nc.dram_tensor("b2c", [D, 1], dt.float32, kind="ExternalInput")
    out_d = nc.dram_tensor("out", [WROWS, D], dt.float32, kind="ExternalOutput")

    # supers: groups of up to 4 windows sharing one [64,512] psum bank
    supers = []
    wi = 0
    while wi < NW:
        sw = min(4, NW - wi)
        supers.append((wi, sw))
        wi += sw

    with tile.TileContext(nc) as tc:
        with (
            tc.tile_pool(name="const", bufs=1) as cpool,
            tc.tile_pool(name="chunks", bufs=2) as chpool,
            tc.tile_pool(name="idxs", bufs=2) as ipool,
            tc.tile_pool(name="mtiles", bufs=8) as mpool,
            tc.tile_pool(name="small", bufs=3) as spool,
            tc.tile_pool(name="psA", bufs=2, space="PSUM") as psA,
            tc.tile_pool(name="psB", bufs=2, space="PSUM") as psB,
            tc.tile_pool(name="psT", bufs=2, space="PSUM") as psT,
            tc.tile_pool(name="dram", bufs=1, space="DRAM") as dpool,
        ):
            # resident SBUF state
            dstloc1_sb = cpool.tile([P, T], dt.float32, tag="dstloc1")
            wts1_sb = cpool.tile([P, T], dt.float32, tag="wts1")
            dstloc2_sb = cpool.tile([P, T2], dt.float32, tag="dstloc2")
            wts2_sb = cpool.tile([P, T2], dt.float32, tag="wts2")
            xT_sb = cpool.tile([D, WROWS], dt.bfloat16, tag="xT")
            iota_sb = cpool.tile([P, P], dt.bfloat16, tag="iota")
            id64_sb = cpool.tile([D, D], dt.bfloat16, tag="id64")
            id64f_sb = cpool.tile([D, D], dt.float32, tag="id64f")
            w1l_sb = cpool.tile([D, D], dt.bfloat16, tag="w1l")
            w1r_sb = cpool.tile([D, D], dt.bfloat16, tag="w1r")
            w2l_sb = cpool.tile([D, D], dt.bfloat16, tag="w2l")
            w2r_sb = cpool.tile([D, D], dt.bfloat16, tag="w2r")
            b1_sb = cpool.tile([D, 1], dt.float32, tag="b1")
            b2_sb = cpool.tile([D, 1], dt.float32, tag="b2")
            h1T_sb = cpool.tile([D, WROWS], dt.bfloat16, tag="h1T")
            h1rows_sb = cpool.tile([P, NW * E2], dt.bfloat16, tag="h1rows")
            out2T_sb = cpool.tile([D, WROWS], dt.bfloat16, tag="out2T")

            for t_sb, t_d in [(dstloc1_sb, dstloc1_d), (wts1_sb, wts1_d),
                              (dstloc2_sb, dstloc2_d), (wts2_sb, wts2_d),
                              (xT_sb, xT_d),
                              (iota_sb, iota_d), (id64_sb, id64_d),
                              (id64f_sb, id64f_d),
                              (w1l_sb, w1l_d), (w1r_sb, w1r_d),
                              (w2l_sb, w2l_d), (w2r_sb, w2r_d),
                              (b1_sb, b1_d), (b2_sb, b2_d)]:
                nc.sync.dma_start(out=t_sb[:], in_=t_d.ap())
            # zero pad halves of the padded row staging once
            nc.vector.memset(h1rows_sb[:], 0.0)

            for _rep in range(reps):
                h1loc_dram = dpool.tile([WROWS, E2], dt.bfloat16,
                                        tag=f"h1loc{_rep}")
                h1full_dram = dpool.tile([TBL_ROWS, E2], dt.bfloat16,
                                         tag=f"h1full{_rep}",
                                         addr_space="Shared")
                # ---------------- layer 1 ----------------
                nchunks1 = (NW + CW1 - 1) // CW1
                chunk_tiles = {}
                for ci in range(nchunks1):
                    w0 = ci * CW1
                    nw = min(CW1, NW - w0)
                    ch = chpool.tile([P, CH_COLS], dt.bfloat16, tag="bigchunk")
                    nc.sync.dma_start(
                        out=ch[:, :nw * K * D],
                        in_=msgs_d.ap()[:, w0 * K * D:(w0 + nw) * K * D])
                    chunk_tiles[ci] = ch

                for w0, sw in supers:
                    agg_ps = psA.tile([D, 4 * P], dt.float32, tag="agg")
                    for s in range(sw):
                        wi = w0 + s
                        ci, woff = wi // CW1, wi % CW1
                        ch = chunk_tiles[ci]
                        for k in range(K):
                            t = wi * K + k
                            if "no_mbuild" in probe:
                                mt = iota_sb
                            else:
                                mt = mpool.tile([P, P], dt.bfloat16, tag="M")
                                nc.vector.tensor_scalar(
                                    out=mt[:], in0=iota_sb[:],
                                    scalar1=dstloc1_sb[:, t:t + 1],
                                    scalar2=wts1_sb[:, t:t + 1],
                                    op0=mybir.AluOpType.is_equal,
                                    op1=mybir.AluOpType.mult)
                            if "no_pe_agg" not in probe:
                                nc.tensor.matmul(
                                    out=agg_ps[:, s * P:(s + 1) * P],
                                    lhsT=ch[:, (woff * K + k) * D:(woff * K + k + 1) * D],
                                    rhs=mt[:], start=(k == 0), stop=(k == K - 1))
                    agg_sb = spool.tile([D, 4 * P], dt.bfloat16, tag="aggsb")
                    nc.vector.tensor_copy(out=agg_sb[:, :sw * P],
                                          in_=agg_ps[:, :sw * P])
                    h_ps = psB.tile([D, 4 * P], dt.float32, tag="hps")
                    nc.tensor.matmul(out=h_ps[:, :sw * P], lhsT=w1l_sb[:],
                                     rhs=agg_sb[:, :sw * P], start=True, stop=False)
                    nc.tensor.matmul(out=h_ps[:, :sw * P], lhsT=w1r_sb[:],
                                     rhs=xT_sb[:, w0 * P:(w0 + sw) * P],
                                     start=False, stop=True)
                    nc.scalar.activation(
                        out=h1T_sb[:, w0 * P:(w0 + sw) * P], in_=h_ps[:, :sw * P],
                        func=mybir.ActivationFunctionType.Relu, bias=b1_sb[:])

                # transpose h1T -> padded rows, stage, DMA to DRAM, AllGather
                for wi in range(NW):
                    tp = psT.tile([P, D], dt.float32, tag="tp")
                    nc.tensor.matmul(out=tp[:], lhsT=h1T_sb[:, wi * P:(wi + 1) * P],
                                     rhs=id64_sb[:], start=True, stop=True)
                    nc.vector.tensor_copy(
                        out=h1rows_sb[:, wi * E2:wi * E2 + D], in_=tp[:])
                nc.sync.dma_start(
                    out=h1loc_dram[:].rearrange("(p t) f -> p (t f)", p=P),
                    in_=h1rows_sb[:])
                if "no_allgather" not in probe:
                    nc.gpsimd.collective_compute(
                        "AllGather", mybir.AluOpType.bypass,
                        replica_groups=[list(range(NCORES))],
                        ins=[h1loc_dram[:]], outs=[h1full_dram[:]])

                # ---------------- layer 2 ----------------
                g_tiles = {}
                for ci in range(nchunks2):
                    ch = chpool.tile([P, CH_COLS], dt.bfloat16, tag="bigchunk")
                    if "no_l2gather" in probe:
                        g_tiles[ci] = ch
                        continue
                    ib = ipool.tile([P, CW2 * sumK2 * 8], dt.int16, tag="idx")
                    c0 = ci * sumK2 * CW2 * 8
                    nc.sync.dma_start(
                        out=ib[:],
                        in_=idx2_d.ap()[:, c0:c0 + CW2 * sumK2 * 8])
                    for qi in range(NQ):
                        nq_blocks = CW2 * K2[qi]
                        gcol0 = pref[qi] * CW2 * GELEM
                        icol0 = pref[qi] * CW2 * 8
                        b0 = 0
                        while b0 < nq_blocks:   # <=1024 idxs per op (ring cap)
                            nb = min(8, nq_blocks - b0)
                            ni = nb * P
                            gview = (ch[:, gcol0 + b0 * GELEM:
                                        gcol0 + (b0 + nb) * GELEM]
                                     .rearrange("p (g e) -> p g e", e=GELEM))
                            tblv = h1full_dram[qi * QROWS:(qi + 1) * QROWS, :]
                            idxv = ib[:, icol0 + b0 * 8:icol0 + (b0 + nb) * 8]
                            if GELEM == E2:
                                nc.gpsimd.dma_gather(
                                    gview, tblv, idxv, ni, ni, E2)
                            else:
                                _gather_direct(nc, mybir, gview, tblv, idxv,
                                               ni, GELEM, E2)
                            b0 += nb
                    g_tiles[ci] = ch

                for w0, sw in supers:
                    agg_ps = psA.tile([D, 4 * P], dt.float32, tag="agg")
                    for s in range(sw):
                        wi = w0 + s
                        ci, wl = wi // CW2, wi % CW2
                        ch = g_tiles[ci]
                        nt = 0
                        for qi in range(NQ):
                            for k in range(K2[qi]):
                                t2 = (ci * sumK2 * CW2 + pref[qi] * CW2
                                      + wl * K2[qi] + k)
                                blk = pref[qi] * CW2 + wl * K2[qi] + k
                                if "no_mbuild" in probe:
                                    mt = iota_sb
                                else:
                                    mt = mpool.tile([P, P], dt.bfloat16, tag="M")
                                    nc.vector.tensor_scalar(
                                        out=mt[:], in0=iota_sb[:],
                                        scalar1=dstloc2_sb[:, t2:t2 + 1],
                                        scalar2=wts2_sb[:, t2:t2 + 1],
                                        op0=mybir.AluOpType.is_equal,
                                        op1=mybir.AluOpType.mult)
                                if "no_pe_agg" not in probe:
                                    nc.tensor.matmul(
                                        out=agg_ps[:, s * P:(s + 1) * P],
                                        lhsT=ch[:, blk * GELEM:blk * GELEM + D],
                                        rhs=mt[:], start=(nt == 0),
                                        stop=(nt == sumK2 - 1))
                                nt += 1
                    agg_sb = spool.tile([D, 4 * P], dt.bfloat16, tag="aggsb")
                    nc.vector.tensor_copy(out=agg_sb[:, :sw * P],
                                          in_=agg_ps[:, :sw * P])
                    h_ps = psB.tile([D, 4 * P], dt.float32, tag="hps")
                    nc.tensor.matmul(out=h_ps[:, :sw * P], lhsT=w2l_sb[:],
                                     rhs=agg_sb[:, :sw * P], start=True, stop=False)
                    nc.tensor.matmul(out=h_ps[:, :sw * P], lhsT=w2r_sb[:],
                                     rhs=h1T_sb[:, w0 * P:(w0 + sw) * P],
                                     start=False, stop=True)
                    nc.vector.tensor_scalar_add(
                        out=out2T_sb[:, w0 * P:(w0 + sw) * P],
                        in0=h_ps[:, :sw * P], scalar1=b2_sb[:])

                # transpose out2T -> rows -> DRAM
                outv = out_d.ap().rearrange("(p t) f -> p t f", p=P)
                for wi in range(NW):
                    tp = psT.tile([P, D], dt.float32, tag="tp")
                    nc.tensor.matmul(out=tp[:], lhsT=out2T_sb[:, wi * P:(wi + 1) * P],
                                     rhs=id64_sb[:], start=True, stop=True)
                    ot = spool.tile([P, D], dt.float32, tag="orow")
                    nc.vector.tensor_copy(out=ot[:], in_=tp[:])
                    nc.sync.dma_start(out=outv[:, wi, :], in_=ot[:])

    nc.compile()
    return nc


PREP_VERSION = 4


def _compute_K(src, dst):
    wid = (dst % NLOC) // P + (dst // NLOC) * NW
    wc = np.bincount(wid, minlength=NCORES * NW)
    K = max(int(np.max((wc + P - 1) // P)), 1)
    q = _layout_row(src) // QROWS
    gid = wid * NQ + q
    gc = np.bincount(gid, minlength=NCORES * NW * NQ).reshape(NCORES * NW, NQ)
    K2 = [max(int(np.max((gc[:, qi] + P - 1) // P)), 1) for qi in range(NQ)]
    return K, K2


def prep_all(inputs):
    """Host-side prep: returns (K, K2, in_maps) for the 8 cores."""
    x = np.asarray(inputs["x"], dtype=np.float32)
    edge_index = np.asarray(inputs["edge_index"])
    src = edge_index[0].astype(np.int64)
    dst = edge_index[1].astype(np.int64)
    cnt = np.bincount(dst, minlength=N).astype(np.float32)
    inv = (1.0 / np.maximum(cnt, 1.0)).astype(np.float32)
    K, K2 = _compute_K(src, dst)

    iota = np.tile(np.arange(P, dtype=np.float32), (P, 1)).astype(BF16)
    id64 = np.eye(D, dtype=np.float32)
    common = {
        "iota": iota, "id64": id64.astype(BF16), "id64f": id64,
        "w1lT": np.asarray(inputs["W1l"], np.float32).T.astype(BF16).copy(),
        "w1rT": np.asarray(inputs["W1r"], np.float32).T.astype(BF16).copy(),
        "w2lT": np.asarray(inputs["W2l"], np.float32).T.astype(BF16).copy(),
        "w2rT": np.asarray(inputs["W2r"], np.float32).T.astype(BF16).copy(),
        "b1c": np.asarray(inputs["b1"], np.float32).reshape(D, 1).copy(),
        "b2c": np.asarray(inputs["b2"], np.float32).reshape(D, 1).copy(),
    }
    in_maps = []
    for c in range(NCORES):
        per = _prep_core(c, src, dst, inv, x, K, K2)
        in_maps.append({**common, **per})
    return K, K2, in_maps


def kernel(x, edge_index, W1l, W1r, b1, W2l, W2r, b2):
    from concourse import bass_utils

    K, K2, in_maps = prep_all(dict(x=x, edge_index=edge_index, W1l=W1l,
                                   W1r=W1r, b1=b1, W2l=W2l, W2r=W2r, b2=b2))
    nc = _build_program(K, K2)
    res = bass_utils.run_bass_kernel_spmd(nc, in_maps, list(range(NCORES)))

    outs = []
    for c in range(NCORES):
        o = res.results[c]["out"]  # [WROWS, 64], row = p*NW + t
        o = o.reshape(P, NW, D).transpose(1, 0, 2).reshape(WROWS, D)[:NLOC]
        outs.append(o)
    return np.concatenate(outs, axis=0).astype(np.float32)
